# revision 16
# baseline (speedup 1.0000x reference)
"""GATv2 attention scores kernel for Trainium2 (8 NeuronCores, Bass/Tile).

Computes attn = softmax_j( sum_d a[h,d] * silu(q[b,h,i,d] + k[b,h,j,d]) )
for q,k: [B,H,N,D] = [16,8,256,32], output [B,H,N,N] f32.

Sharding: the 128 (b,h) pairs are data-parallel; each of the 8 cores
handles 16 pairs. No collectives.

Algorithm — cosine-series factorization of the GATv2 score:
  silu(x) = 0.5*x + h(x) with h(x) = 0.5*x*tanh(x/2) even, and
  h(x) ~= C + sum_{c=1..4} A_c cos(w_c x)  (free-frequency weighted LSQ
  fit over the N(0,2) input distribution, wrms ~6e-4).
  cos(w(u+v)) = cos(wu)cos(wv) - sin(wu)sin(wv)  turns the N^2*D
  elementwise silu into a rank-9 matmul over sin/cos features:

    scores[i,j] ~= const(i) + 0.5*sum_d a_d k_jd
                 + sum_{c,d} a_d A_c [cos(w_c q_id)cos(w_c k_jd)
                                      - sin(w_c q_id)sin(w_c k_jd)]

  const(i) terms (0.5 a.q_i and C sum a) are dropped: softmax over j is
  invariant to per-row constants. The sin-side minus is folded into the
  host-negated sin-q args (sin is odd). The linear beta_j block needs no
  extra input: mode-0 k-args are w_0*k unwrapped (|w_0 k| < pi always),
  so a constant per-head lhsT of 0.5*a_d/w_0 against the raw mode-0
  k-arg rows reproduces 0.5*sum_d a_d k_jd on the PE.

Per-core pipeline, two pairs ("duo") per step:
  - Host packs one fp16 blob [128, 2048] per duo: per pair four 256-col
    groups of sin/cos args (w_c*x + phase, range-reduced into [-pi,pi] —
    the ACT Sin table is only valid there) at partition 32c+d.
  - ACT: one Sin op [128,2048] -> all features fp16 (duo 0 is split in
    two pair-level DMAs + Sin ops so ACT starts sooner).
  - DVE: one tensor_scalar_mul (4x mode) per pair folds a_d*A_c into the
    q-side features (per-head [128,1] column of a resident wvec).
  - PE: per pair x output-half, 3 chained fp16 matmuls (sin, cos,
    linear) accumulate scores into a [128,1024] PSUM tile (2 banks per
    duo; a group of 4 duos fills all 8 banks).
  - ACT table discipline: duos run in two groups of 4 with nosync gate
    edges ordering the ACT stream [sins A][exps A][sins B][exps B], so
    only 4 LoadActFuncSet (1283ns) are inserted, no PSUM spill copies
    are needed (exp always reads PSUM), and group A's softmax tail
    overlaps group B's sins.
  - ACT: Exp [128,1024] fp16 out, bias -2 (softmax-invariant) so fp16
    row sums stay well under 65504; DVE: one 3D-AP row-sum reduce ->
    [128,4] fp16 + reciprocal; normalize muls split DVE (4x mode) /
    Pool; one 4D-AP DMA per duo writes fp16 out (host upcasts to f32).

mask is all-False for this problem (spec fill=zeros): if a nonzero mask
is ever passed, an exact host-side renormalization fallback is applied.
scale is unused by the module.
"""

import os
import numpy as np
from contextlib import ExitStack

import concourse.bass as bass
import concourse.bacc as bacc
import concourse.mybir as mybir
import concourse.tile as tile
import bass_rust as _bass_rust
from concourse.bass_utils import run_bass_kernel_spmd

B, H, N, D = 16, 8, 256, 32
NCORES = 8
PAIRS = (B * H) // NCORES      # 16 (b,h) pairs per core
DUOS = PAIRS // 2
BH = B * H

FP16 = mybir.dt.float16
FP32 = mybir.dt.float32

# cosine-series fit of h(x) = silu(x) - 0.5x on [-12,12], weight
# exp(-x^2/4) + 1e-4 (x = q+k ~ N(0,2)); constant term dropped (softmax)
OMEGA = np.array([0.25583485, 0.73377396, 1.22431455, 1.93659498])
AMP = np.array([-2.62677989, -0.30220448, -0.07415507, -0.01321925])

INP_BUFS = int(os.environ.get("GAT_INP_BUFS", "8"))
F_BUFS = int(os.environ.get("GAT_F_BUFS", "4"))
W_BUFS = int(os.environ.get("GAT_W_BUFS", "4"))
X_BUFS = int(os.environ.get("GAT_X_BUFS", "5"))
S_BUFS = int(os.environ.get("GAT_S_BUFS", "6"))
R_BUFS = int(os.environ.get("GAT_R_BUFS", "4"))
GROUP = int(os.environ.get("GAT_GROUP", "4"))     # duos per act-table group
DVE_MULS = int(os.environ.get("GAT_DVE_MULS", "2"))  # of 4 norm muls on DVE
FINE_DUOS = int(os.environ.get("GAT_FINE_DUOS", "2"))  # pair-level tail duos

_cache = {}


def build_program() -> bacc.Bacc:
    if "nc" in _cache:
        return _cache["nc"]
    nc = bacc.Bacc("TRN2")
    blob_d = nc.declare_dram_parameter("blob", [DUOS, 128, 2048], FP16,
                                       isOutput=False)
    wvec_d = nc.declare_dram_parameter("wvec", [128, H + 1], FP32,
                                       isOutput=False)
    lhc_d = nc.declare_dram_parameter("lhc", [32, H * 128], FP16,
                                      isOutput=False)
    out_d = nc.declare_dram_parameter("out", [PAIRS, N, N], FP16,
                                      isOutput=True)

    Sin = mybir.ActivationFunctionType.Sin
    Exp = mybir.ActivationFunctionType.Exp

    with ExitStack() as ctx:
        tc = ctx.enter_context(tile.TileContext(nc))
        cpool = ctx.enter_context(tc.tile_pool(name="cpool", bufs=1))
        inp = ctx.enter_context(tc.tile_pool(name="inp", bufs=INP_BUFS))
        fpool = ctx.enter_context(tc.tile_pool(name="fpool", bufs=F_BUFS))
        wpool = ctx.enter_context(tc.tile_pool(name="wpool", bufs=2 * W_BUFS))
        ppool = ctx.enter_context(
            tc.tile_pool(name="ppool", bufs=4, space="PSUM"))
        dpool = ctx.enter_context(tc.tile_pool(name="dpool", bufs=4))
        xpool = ctx.enter_context(tc.tile_pool(name="xpool", bufs=X_BUFS))
        spool = ctx.enter_context(tc.tile_pool(name="spool", bufs=2 * S_BUFS))
        rpool = ctx.enter_context(tc.tile_pool(name="rpool", bufs=R_BUFS))

        # DMA order: blob-0's first half (gates sin-0) leads, then the
        # small consts (wvec gates ALL q-scales and thus the whole PE
        # stream — it must land early), then the remaining blobs
        bl_tiles = []
        for t in range(DUOS):
            bl = inp.tile([128, 2048], FP16, tag="bl")
            if t == 0:
                nc.sync.dma_start(bl[:, 0:1024], blob_d[0, :, 0:1024])
                wv = cpool.tile([128, H + 1], FP32, name="wv", tag="wv")
                nc.sync.dma_start(wv[:], wvec_d[:])
                lhc = cpool.tile([32, H * 128], FP16, name="lhc", tag="lhc")
                nc.sync.dma_start(lhc[:], lhc_d[:])
                nc.sync.dma_start(bl[:, 1024:2048], blob_d[0, :, 1024:2048])
            else:
                nc.sync.dma_start(bl[:], blob_d[t])
            bl_tiles.append(bl)

        def phase1(t):
            """sin + q-scale + matmuls for duo t; returns (sin_insts, P)."""
            bl = bl_tiles[t]
            sins = []
            F = fpool.tile([128, 2048], FP16, tag="f")
            if t == 0:
                sins.append(
                    nc.scalar.activation(F[:, 0:1024], bl[:, 0:1024], Sin))
                sins.append(
                    nc.scalar.activation(F[:, 1024:2048], bl[:, 1024:2048],
                                         Sin))
            else:
                sins.append(nc.scalar.activation(F[:], bl[:, 0:2048], Sin))
            P = ppool.tile([128, 1024], FP32, tag="ps")
            for s in (0, 1):
                h = (2 * t + s) % H
                base = 1024 * s
                QW = wpool.tile([128, 512], FP16, tag="qw")
                nc.vector.tensor_scalar_mul(QW[:], F[:, base:base + 512],
                                            wv[:, h:h + 1])
                for ih in (0, 1):
                    o = P[:, 512 * s + 256 * ih:512 * s + 256 * ih + 256]
                    lo = 128 * ih
                    # -sin(wq)w . sin(wk)  (minus folded into args)
                    nc.tensor.matmul(o, QW[:, lo:lo + 128],
                                     F[:, base + 512:base + 768],
                                     start=True, stop=False,
                                     skip_group_check=True)
                    # cos(wq)w . cos(wk)
                    nc.tensor.matmul(o, QW[:, 256 + lo:256 + lo + 128],
                                     F[:, base + 768:base + 1024],
                                     start=False, stop=False,
                                     skip_group_check=True)
                    # linear beta_j from raw mode-0 k-args (= w0*k):
                    # lhsT = 0.5*a_d/w0 const cols, contraction K=32
                    nc.tensor.matmul(o, lhc[:, 128 * h:128 * h + 128],
                                     bl[0:32, base + 512:base + 768],
                                     start=False, stop=True,
                                     skip_group_check=True)
            return sins, P

        def exp_sums(t, src, gate):
            """exp (ordered after gate) + row sums + reciprocal."""
            X = xpool.tile([128, 1024], FP16, tag="x")
            # bias -2 (softmax-invariant) keeps fp16 row sums well under
            # 65504 (observed max ~25k unshifted -> ~3.4k)
            ei = nc.scalar.activation(X[:], src[:], Exp, bias=wv[:, H:H + 1])
            _bass_rust.add_dep_helper(
                ei.ins, gate.ins, sync=False,
                reason="act-table grouping: exps after sins")
            S4 = spool.tile([128, 4], FP16, tag="s")
            R4 = spool.tile([128, 4], FP32, tag="r4")
            with nc.allow_low_precision("fp16 row sums: denominators only, "
                                        "reduce accumulates wide"):
                nc.vector.reduce_sum(
                    S4[:], X[:].rearrange("p (g j) -> p g j", g=4),
                    axis=mybir.AxisListType.X)
            nc.vector.reciprocal(R4[:], S4[:])
            return ei, X, R4

        def norm_store(t, X, R4, last):
            """normalize muls (DVE/Pool split) + output DMA."""
            dve_muls = 4 if last else DVE_MULS
            RN = rpool.tile([128, 1024], FP16, tag="rn")
            for g in range(4):
                if g < dve_muls:
                    nc.vector.tensor_scalar_mul(RN[:, 256 * g:256 * g + 256],
                                                X[:, 256 * g:256 * g + 256],
                                                R4[:, g:g + 1])
                else:
                    nc.gpsimd.tensor_scalar_mul(RN[:, 256 * g:256 * g + 256],
                                                X[:, 256 * g:256 * g + 256],
                                                R4[:, g:g + 1])
            if last:
                # per-pair stores so the final transfer is small
                for s in (0, 1):
                    dst = out_d[2 * t + s].rearrange("(h i) j -> i h j", h=2)
                    nc.sync.dma_start(dst, RN[:, 512 * s:512 * s + 512])
            else:
                dst = out_d[2 * t:2 * t + 2].rearrange(
                    "p (h i) j -> i p h j", h=2)
                nc.sync.dma_start(dst, RN[:])

        # act-table groups: [sins A][exps A][sins B][exps B]...
        gate = None
        for g0 in range(0, DUOS, GROUP):
            group = list(range(g0, min(g0 + GROUP, DUOS)))
            sins_all = []
            src = {}
            for t in group:
                sins, P = phase1(t)
                if gate is not None:
                    # order this group's sins after the previous group's
                    # last exp so the scheduler keeps table switches rare
                    for si in sins:
                        _bass_rust.add_dep_helper(
                            si.ins, gate.ins, sync=False,
                            reason="act-table grouping: sins after exps")
                sins_all.extend(sins)
                if len(group) > 4 and t - g0 < len(group) - 4:
                    # more than 4 duos in flight: exit early scores
                    # PSUM -> SBUF (DVE fp16 bitcast) to free banks
                    Dn = dpool.tile([128, 1024], FP32, tag="dn")
                    nc.vector.tensor_copy(Dn[:].bitcast(FP16),
                                          P[:].bitcast(FP16))
                    src[t] = Dn
                else:
                    src[t] = P
            sgate = sins_all[-1]
            is_last_group = g0 + GROUP >= DUOS
            n_fine = min(FINE_DUOS, len(group)) if is_last_group else 0
            for t in group:
                if t >= group[-1] - n_fine + 1 and n_fine:
                    # tail duos: per-pair exp/reduce/normalize/store so
                    # the post-last-exp chain is half-length
                    X = xpool.tile([128, 1024], FP16, tag="x")
                    S4 = spool.tile([128, 4], FP16, tag="s")
                    R4 = spool.tile([128, 4], FP32, tag="r4")
                    RN = rpool.tile([128, 1024], FP16, tag="rn")
                    for s in (0, 1):
                        hs = slice(512 * s, 512 * s + 512)
                        ei = nc.scalar.activation(X[:, hs], src[t][:, hs],
                                                  Exp, bias=wv[:, H:H + 1])
                        _bass_rust.add_dep_helper(
                            ei.ins, sgate.ins, sync=False,
                            reason="act-table grouping: exps after sins")
                        with nc.allow_low_precision("fp16 row sums"):
                            nc.vector.reduce_sum(
                                S4[:, 2 * s:2 * s + 2],
                                X[:, hs].rearrange("p (g j) -> p g j", g=2),
                                axis=mybir.AxisListType.X)
                        nc.vector.reciprocal(R4[:, 2 * s:2 * s + 2],
                                             S4[:, 2 * s:2 * s + 2])
                        for g in (2 * s, 2 * s + 1):
                            nc.vector.tensor_scalar_mul(
                                RN[:, 256 * g:256 * g + 256],
                                X[:, 256 * g:256 * g + 256], R4[:, g:g + 1])
                        dst = out_d[2 * t + s].rearrange("(h i) j -> i h j",
                                                         h=2)
                        nc.sync.dma_start(dst, RN[:, hs])
                else:
                    ei, X, R4 = exp_sums(t, src[t], sgate)
                    norm_store(t, X, R4, last=False)
            gate = ei

    nc.compile()
    _cache["nc"] = nc
    return nc


def prepare_in_maps(q, k, attention):
    q = np.asarray(q, dtype=np.float32).reshape(BH, N, D)
    k = np.asarray(k, dtype=np.float32).reshape(BH, N, D)
    a = np.asarray(attention, dtype=np.float32).reshape(H, D)

    qT = q.transpose(0, 2, 1)          # [BH, D, N]
    kT = k.transpose(0, 2, 1)
    aq = OMEGA[None, :, None, None] * qT[:, None, :, :]   # [BH, 4, D, N]
    ak = OMEGA[None, :, None, None] * kT[:, None, :, :]

    def wrap(x):
        # range-reduce into [-pi, pi]: ACT Sin is only accurate there
        return (x + np.pi) % (2 * np.pi) - np.pi

    args = np.concatenate([
        wrap(-aq).reshape(BH, 128, N),             # -> -sin(wq)
        wrap(aq + np.pi / 2).reshape(BH, 128, N),  # -> cos(wq)
        wrap(ak).reshape(BH, 128, N),              # -> sin(wk)
        wrap(ak + np.pi / 2).reshape(BH, 128, N),  # -> cos(wk)
    ], axis=2)                                     # [BH, 128, 1024]
    blob = args.reshape(BH // 2, 2, 128, 1024).transpose(
        0, 2, 1, 3).reshape(BH // 2, 128, 2048).astype(np.float16)

    wvec = np.zeros((128, H + 1), np.float32)
    wvec[:, H] = -2.0      # exp bias: softmax-invariant shift for fp16 sums
    for hd in range(H):
        for c in range(4):
            wvec[32 * c:32 * c + 32, hd] = a[hd] * AMP[c]
    # lhc[d, 128h:128(h+1)] = 0.5 * a[h,d] / w0  (beta_j via mode-0 k-args)
    lhc = np.zeros((32, H * 128), np.float16)
    for hd in range(H):
        lhc[:, 128 * hd:128 * hd + 128] = (
            0.5 * a[hd] / OMEGA[0])[:, None].astype(np.float16)

    in_maps = []
    for cix in range(NCORES):
        s = slice(cix * DUOS, (cix + 1) * DUOS)
        in_maps.append({
            "blob": np.ascontiguousarray(blob[s]),
            "wvec": wvec,
            "lhc": lhc,
        })
    return in_maps


def unshard_output(results) -> np.ndarray:
    outs = [np.asarray(r["out"]) for r in results]
    return np.concatenate(outs, axis=0).reshape(B, H, N, N).astype(np.float32)


def kernel(q, k, scale, mask, attention) -> np.ndarray:
    nc = build_program()
    in_maps = prepare_in_maps(q, k, attention)
    res = run_bass_kernel_spmd(nc, in_maps, list(range(NCORES)))
    attn = unshard_output(res.results)
    mask = np.asarray(mask)
    if mask.any():
        # exact post-hoc masking: softmax with -inf masked scores equals
        # zeroing masked probabilities and renormalizing
        keep = ~np.broadcast_to(mask, attn.shape)
        kept = attn * keep
        denom = kept.sum(-1, keepdims=True)
        nkeep = keep.sum(-1, keepdims=True)
        uniform = np.where(nkeep > 0, keep / np.maximum(nkeep, 1), 1.0 / N)
        attn = np.where(denom > 0, kept / np.maximum(denom, 1e-38), uniform)
        attn = attn.astype(np.float32)
    return attn


# revision 23
# speedup vs baseline: 1.0300x; 1.0300x over previous
"""GATv2 attention scores kernel for Trainium2 (8 NeuronCores, Bass/Tile).

Computes attn = softmax_j( sum_d a[h,d] * silu(q[b,h,i,d] + k[b,h,j,d]) )
for q,k: [B,H,N,D] = [16,8,256,32], output [B,H,N,N] f32.

Sharding: the 128 (b,h) pairs are data-parallel; each of the 8 cores
handles 16 pairs. No collectives.

Algorithm — cosine-series factorization of the GATv2 score:
  silu(x) = 0.5*x + h(x) with h(x) = 0.5*x*tanh(x/2) even, and
  h(x) ~= C + sum_{c=1..4} A_c cos(w_c x)  (free-frequency weighted LSQ
  fit over the N(0,2) input distribution, wrms ~6e-4).
  cos(w(u+v)) = cos(wu)cos(wv) - sin(wu)sin(wv)  turns the N^2*D
  elementwise silu into a rank-9 matmul over sin/cos features:

    scores[i,j] ~= const(i) + 0.5*sum_d a_d k_jd
                 + sum_{c,d} a_d A_c [cos(w_c q_id)cos(w_c k_jd)
                                      - sin(w_c q_id)sin(w_c k_jd)]

  const(i) terms (0.5 a.q_i and C sum a) are dropped: softmax over j is
  invariant to per-row constants. The sin-side minus is folded into the
  host-negated sin-q args (sin is odd). The linear beta_j block needs no
  extra input: mode-0 k-args are w_0*k unwrapped (|w_0 k| < pi always),
  so a constant per-head lhsT of 0.5*a_d/w_0 against the raw mode-0
  k-arg rows reproduces 0.5*sum_d a_d k_jd on the PE.

Per-core pipeline, two pairs ("duo") per step:
  - Host packs one fp16 blob [128, 2048] per duo: per pair four 256-col
    groups of sin/cos args (w_c*x + phase, range-reduced into [-pi,pi] —
    the ACT Sin table is only valid there) at partition 32c+d.
  - ACT: one Sin op [128,2048] -> all features fp16 (duo 0 is split in
    two pair-level DMAs + Sin ops so ACT starts sooner).
  - DVE: one tensor_scalar_mul (4x mode) per pair folds a_d*A_c into the
    q-side features (per-head [128,1] column of a resident wvec).
  - PE: per pair x output-half, 3 chained fp16 matmuls (sin, cos,
    linear) accumulate scores into a [128,1024] PSUM tile (2 banks per
    duo; a group of 4 duos fills all 8 banks).
  - ACT table discipline: duos run in two groups of 4 with nosync gate
    edges ordering the ACT stream [sins A][exps A][sins B][exps B], so
    only 4 LoadActFuncSet (1283ns) are inserted, no PSUM spill copies
    are needed (exp always reads PSUM), and group A's softmax tail
    overlaps group B's sins.
  - ACT: Exp [128,1024] fp16 out, bias -2 (softmax-invariant) so fp16
    row sums stay well under 65504; DVE: one 3D-AP row-sum reduce ->
    [128,4] fp16 + reciprocal; normalize muls split DVE (4x mode) /
    Pool; one 4D-AP DMA per duo writes fp16 out (host upcasts to f32).

mask is all-False for this problem (spec fill=zeros): if a nonzero mask
is ever passed, an exact host-side renormalization fallback is applied.
scale is unused by the module.
"""

import os
import numpy as np
from contextlib import ExitStack

import concourse.bass as bass
import concourse.bacc as bacc
import concourse.mybir as mybir
import concourse.tile as tile
import bass_rust as _bass_rust
from concourse.bass_utils import run_bass_kernel_spmd

B, H, N, D = 16, 8, 256, 32
NCORES = 8
PAIRS = (B * H) // NCORES      # 16 (b,h) pairs per core
DUOS = PAIRS // 2
BH = B * H

FP16 = mybir.dt.float16
FP32 = mybir.dt.float32

# cosine-series fit of h(x) = silu(x) - 0.5x on [-12,12], weight
# exp(-x^2/4) + 1e-4 (x = q+k ~ N(0,2)); constant term dropped (softmax)
OMEGA = np.array([0.25583485, 0.73377396, 1.22431455, 1.93659498])
AMP = np.array([-2.62677989, -0.30220448, -0.07415507, -0.01321925])

INP_BUFS = int(os.environ.get("GAT_INP_BUFS", "8"))
F_BUFS = int(os.environ.get("GAT_F_BUFS", "4"))
W_BUFS = int(os.environ.get("GAT_W_BUFS", "4"))
X_BUFS = int(os.environ.get("GAT_X_BUFS", "6"))
S_BUFS = int(os.environ.get("GAT_S_BUFS", "8"))
R_BUFS = int(os.environ.get("GAT_R_BUFS", "8"))
GROUP = int(os.environ.get("GAT_GROUP", "8"))     # duos per act-table group
DVE_MULS = int(os.environ.get("GAT_DVE_MULS", "2"))  # of 4 norm muls on DVE
ACC_DUOS = int(os.environ.get("GAT_ACC_DUOS", "0"))   # accum-exp tail duos

_cache = {}


def build_program() -> bacc.Bacc:
    if "nc" in _cache:
        return _cache["nc"]
    nc = bacc.Bacc("TRN2")
    blob_d = nc.declare_dram_parameter("blob", [DUOS, 128, 2048], FP16,
                                       isOutput=False)
    wvec_d = nc.declare_dram_parameter("wvec", [128, H + 1], FP32,
                                       isOutput=False)
    lhc_d = nc.declare_dram_parameter("lhc", [32, H * 128], FP16,
                                      isOutput=False)
    out_d = nc.declare_dram_parameter("out", [PAIRS, N, N], FP16,
                                      isOutput=True)

    Sin = mybir.ActivationFunctionType.Sin
    Exp = mybir.ActivationFunctionType.Exp

    with ExitStack() as ctx:
        tc = ctx.enter_context(tile.TileContext(nc))
        cpool = ctx.enter_context(tc.tile_pool(name="cpool", bufs=1))
        inp = ctx.enter_context(tc.tile_pool(name="inp", bufs=INP_BUFS))
        fpool = ctx.enter_context(tc.tile_pool(name="fpool", bufs=F_BUFS))
        wpool = ctx.enter_context(tc.tile_pool(name="wpool", bufs=2 * W_BUFS))
        ppool = ctx.enter_context(
            tc.tile_pool(name="ppool", bufs=4, space="PSUM"))
        dpool = ctx.enter_context(tc.tile_pool(name="dpool", bufs=4))
        xpool = ctx.enter_context(tc.tile_pool(name="xpool", bufs=X_BUFS))
        spool = ctx.enter_context(tc.tile_pool(name="spool", bufs=2 * S_BUFS))
        rpool = ctx.enter_context(tc.tile_pool(name="rpool", bufs=R_BUFS))

        # DMA order: blob-0's first half (gates sin-0) leads, then the
        # small consts (wvec gates ALL q-scales and thus the whole PE
        # stream — it must land early), then the remaining blobs
        bl_tiles = []
        for t in range(DUOS):
            bl = inp.tile([128, 2048], FP16, tag="bl")
            if t == 0:
                nc.sync.dma_start(bl[:, 0:1024], blob_d[0, :, 0:1024])
                nc.sync.dma_start(bl[:, 1024:2048], blob_d[0, :, 1024:2048])
                wv = cpool.tile([128, H + 1], FP32, name="wv", tag="wv")
                nc.sync.dma_start(wv[:], wvec_d[:])
                lhc = cpool.tile([32, H * 128], FP16, name="lhc", tag="lhc")
                nc.sync.dma_start(lhc[:], lhc_d[:])
            else:
                nc.sync.dma_start(bl[:], blob_d[t])
            bl_tiles.append(bl)

        def phase1(t):
            """sin + q-scale + matmuls for duo t; returns (sin_insts, P)."""
            bl = bl_tiles[t]
            sins = []
            F = fpool.tile([128, 2048], FP16, tag="f")
            if t == 0:
                sins.append(
                    nc.scalar.activation(F[:, 0:1024], bl[:, 0:1024], Sin))
                sins.append(
                    nc.scalar.activation(F[:, 1024:2048], bl[:, 1024:2048],
                                         Sin))
            else:
                sins.append(nc.scalar.activation(F[:], bl[:, 0:2048], Sin))
            P = ppool.tile([128, 1024], FP32, tag="ps")
            QWs = []
            for s in (0, 1):
                h = (2 * t + s) % H
                QW = wpool.tile([128, 512], FP16, tag="qw")
                nc.vector.tensor_scalar_mul(QW[:], F[:, 1024 * s:1024 * s + 512],
                                            wv[:, h:h + 1])
                QWs.append(QW)
            for s in (0, 1):
                h = (2 * t + s) % H
                base = 1024 * s
                QW = QWs[s]
                for ih in (0, 1):
                    o = P[:, 512 * s + 256 * ih:512 * s + 256 * ih + 256]
                    lo = 128 * ih
                    # -sin(wq)w . sin(wk)  (minus folded into args)
                    nc.tensor.matmul(o, QW[:, lo:lo + 128],
                                     F[:, base + 512:base + 768],
                                     start=True, stop=False,
                                     skip_group_check=True)
                    # cos(wq)w . cos(wk)
                    nc.tensor.matmul(o, QW[:, 256 + lo:256 + lo + 128],
                                     F[:, base + 768:base + 1024],
                                     start=False, stop=False,
                                     skip_group_check=True)
                    # linear beta_j from raw mode-0 k-args (= w0*k):
                    # lhsT = 0.5*a_d/w0 const cols, contraction K=32
                    nc.tensor.matmul(o, lhc[:, 128 * h:128 * h + 128],
                                     bl[0:32, base + 512:base + 768],
                                     start=False, stop=True,
                                     skip_group_check=True)
            return sins, P

        def exp_sums(t, src, gate):
            """exp (ordered after gate) + row sums + reciprocal."""
            X = xpool.tile([128, 1024], FP16, tag="x")
            # bias -2 (softmax-invariant) keeps fp16 row sums well under
            # 65504 (observed max ~25k unshifted -> ~3.4k)
            ei = nc.scalar.activation(X[:], src[:], Exp, bias=wv[:, H:H + 1])
            _bass_rust.add_dep_helper(
                ei.ins, gate.ins, sync=False,
                reason="act-table grouping: exps after sins")
            S4 = spool.tile([128, 4], FP16, tag="s")
            R4 = spool.tile([128, 4], FP32, tag="r4")
            with nc.allow_low_precision("fp16 row sums: denominators only, "
                                        "reduce accumulates wide"):
                nc.vector.reduce_sum(
                    S4[:], X[:].rearrange("p (g j) -> p g j", g=4),
                    axis=mybir.AxisListType.X)
            nc.vector.reciprocal(R4[:], S4[:])
            return ei, X, R4

        def norm_store(t, X, R4, last):
            """normalize muls (DVE/Pool split) + output DMA. The last
            duo gets all-DVE muls (127ns vs Pool ~450ns) and per-pair
            stores: its chain is the kernel's tail."""
            dve_muls = 4 if last else DVE_MULS
            RN = rpool.tile([128, 1024], FP16, tag="rn")
            for g in range(4):
                if g < dve_muls:
                    nc.vector.tensor_scalar_mul(RN[:, 256 * g:256 * g + 256],
                                                X[:, 256 * g:256 * g + 256],
                                                R4[:, g:g + 1])
                else:
                    nc.gpsimd.tensor_scalar_mul(RN[:, 256 * g:256 * g + 256],
                                                X[:, 256 * g:256 * g + 256],
                                                R4[:, g:g + 1])
            if last:
                # per-pair stores so the final transfer is small
                for s in (0, 1):
                    dst = out_d[2 * t + s].rearrange("(h i) j -> i h j", h=2)
                    nc.sync.dma_start(dst, RN[:, 512 * s:512 * s + 512])
            else:
                dst = out_d[2 * t:2 * t + 2].rearrange(
                    "p (h i) j -> i p h j", h=2)
                nc.sync.dma_start(dst, RN[:])

        # act-table groups: [sins A][exps A][sins B][exps B]...
        gate = None
        for g0 in range(0, DUOS, GROUP):
            group = list(range(g0, min(g0 + GROUP, DUOS)))
            sins_all = []
            src = {}
            for t in group:
                sins, P = phase1(t)
                if gate is not None:
                    # order this group's sins after the previous group's
                    # last exp so the scheduler keeps table switches rare
                    for si in sins:
                        _bass_rust.add_dep_helper(
                            si.ins, gate.ins, sync=False,
                            reason="act-table grouping: sins after exps")
                sins_all.extend(sins)
                if len(group) > 4 and t - g0 < len(group) - 4:
                    # more than 4 duos in flight: exit early scores
                    # PSUM -> SBUF (DVE fp16 bitcast) to free banks
                    Dn = dpool.tile([128, 1024], FP32, tag="dn")
                    nc.vector.tensor_copy(Dn[:].bitcast(FP16),
                                          P[:].bitcast(FP16))
                    src[t] = Dn
                else:
                    src[t] = P
            sgate = sins_all[-1]
            is_last_group = g0 + GROUP >= DUOS
            n_acc = min(ACC_DUOS, len(group)) if is_last_group else 0
            for t in group:
                if t >= group[-1] - n_acc + 1 and n_acc:
                    # tail duos: per-256-group exp WITH accum_out — ACT
                    # emits the row sums itself, so nothing waits on the
                    # (backlogged) DVE reduce lane at the very end
                    X = xpool.tile([128, 1024], FP16, tag="x")
                    SA = spool.tile([128, 4], FP32, tag="sa")
                    R4 = spool.tile([128, 4], FP32, tag="r4")
                    RN = rpool.tile([128, 1024], FP16, tag="rn")
                    for g in range(4):
                        gs = slice(256 * g, 256 * g + 256)
                        ei = nc.scalar.activation(X[:, gs], src[t][:, gs],
                                                  Exp, bias=wv[:, H:H + 1],
                                                  accum_out=SA[:, g:g + 1])
                        _bass_rust.add_dep_helper(
                            ei.ins, sgate.ins, sync=False,
                            reason="act-table grouping: exps after sins")
                        nc.vector.reciprocal(R4[:, g:g + 1], SA[:, g:g + 1])
                        nc.vector.tensor_scalar_mul(RN[:, gs], X[:, gs],
                                                    R4[:, g:g + 1])
                        s, ih = g >> 1, g & 1
                        nc.sync.dma_start(
                            out_d[2 * t + s, 128 * ih:128 * ih + 128, :],
                            RN[:, gs])
                else:
                    ei, X, R4 = exp_sums(t, src[t], sgate)
                    norm_store(t, X, R4,
                               last=(is_last_group and t == group[-1]))
            gate = ei

    nc.compile()
    _cache["nc"] = nc
    return nc


def prepare_in_maps(q, k, attention):
    q = np.asarray(q, dtype=np.float32).reshape(BH, N, D)
    k = np.asarray(k, dtype=np.float32).reshape(BH, N, D)
    a = np.asarray(attention, dtype=np.float32).reshape(H, D)

    qT = q.transpose(0, 2, 1)          # [BH, D, N]
    kT = k.transpose(0, 2, 1)
    aq = OMEGA[None, :, None, None] * qT[:, None, :, :]   # [BH, 4, D, N]
    ak = OMEGA[None, :, None, None] * kT[:, None, :, :]

    def wrap(x):
        # range-reduce into [-pi, pi]: ACT Sin is only accurate there
        return (x + np.pi) % (2 * np.pi) - np.pi

    args = np.concatenate([
        wrap(-aq).reshape(BH, 128, N),             # -> -sin(wq)
        wrap(aq + np.pi / 2).reshape(BH, 128, N),  # -> cos(wq)
        wrap(ak).reshape(BH, 128, N),              # -> sin(wk)
        wrap(ak + np.pi / 2).reshape(BH, 128, N),  # -> cos(wk)
    ], axis=2)                                     # [BH, 128, 1024]
    blob = args.reshape(BH // 2, 2, 128, 1024).transpose(
        0, 2, 1, 3).reshape(BH // 2, 128, 2048).astype(np.float16)

    wvec = np.zeros((128, H + 1), np.float32)
    wvec[:, H] = -2.0      # exp bias: softmax-invariant shift for fp16 sums
    for hd in range(H):
        for c in range(4):
            wvec[32 * c:32 * c + 32, hd] = a[hd] * AMP[c]
    # lhc[d, 128h:128(h+1)] = 0.5 * a[h,d] / w0  (beta_j via mode-0 k-args)
    lhc = np.zeros((32, H * 128), np.float16)
    for hd in range(H):
        lhc[:, 128 * hd:128 * hd + 128] = (
            0.5 * a[hd] / OMEGA[0])[:, None].astype(np.float16)

    in_maps = []
    for cix in range(NCORES):
        s = slice(cix * DUOS, (cix + 1) * DUOS)
        in_maps.append({
            "blob": np.ascontiguousarray(blob[s]),
            "wvec": wvec,
            "lhc": lhc,
        })
    return in_maps


def unshard_output(results) -> np.ndarray:
    outs = [np.asarray(r["out"]) for r in results]
    return np.concatenate(outs, axis=0).reshape(B, H, N, N).astype(np.float32)


def kernel(q, k, scale, mask, attention) -> np.ndarray:
    nc = build_program()
    in_maps = prepare_in_maps(q, k, attention)
    res = run_bass_kernel_spmd(nc, in_maps, list(range(NCORES)))
    attn = unshard_output(res.results)
    mask = np.asarray(mask)
    if mask.any():
        # exact post-hoc masking: softmax with -inf masked scores equals
        # zeroing masked probabilities and renormalizing
        keep = ~np.broadcast_to(mask, attn.shape)
        kept = attn * keep
        denom = kept.sum(-1, keepdims=True)
        nkeep = keep.sum(-1, keepdims=True)
        uniform = np.where(nkeep > 0, keep / np.maximum(nkeep, 1), 1.0 / N)
        attn = np.where(denom > 0, kept / np.maximum(denom, 1e-38), uniform)
        attn = attn.astype(np.float32)
    return attn
